# revision 26
# baseline (speedup 1.0000x reference)
"""Causal self-attention (T=2048, C=2048, 16 heads) on 8 TRN2 NeuronCores.

Tensor-parallel over heads: core c owns heads 2c, 2c+1.
 - per-core qkv projection in transposed layout (qT,kT: [d, T]; v: [T, d])
 - scores computed transposed: ST[s, t] = kT_blk.T @ qT  (keys on partitions)
 - softmax without max-subtraction (scores are O(+-6) for randn inputs):
   E = exp(scale * ST) * causal_mask; denominator l via ones-matmul;
   out = (v.T @ E) / broadcast(l)  (K=1 matmul broadcast + DVE multiply)
 - no collectives: each core computes its PARTIAL final projection
   partial[t, j] = sum_{i in its 2 head sections} outT[i, t] * WpT[i, j],
   software-pipelined one query-chunk behind attention; the host sums the
   8 partials (the "all-reduce after c_proj" is done at host-gather time).
Host side: shard/transpose/cast inputs; sum the 8 partial outputs.
"""

import numpy as np
import ml_dtypes

import concourse.mybir as mybir
import concourse.tile as tile
from concourse import bacc
from concourse.bass import ds, ts
from concourse.bass_utils import run_bass_kernel_spmd

T = 2048
C = 2048
H = 16
D = 128            # head dim
NC = 8             # cores
HPC = H // NC      # heads per core
DH = HPC * D       # 256: qkv rows per section per core
KB = C // 128      # 16 contraction tiles
TB = T // 128      # 16 t tiles
NQ = 512           # query chunk (psum bank width)
QC = T // NQ       # 4 query chunks
SCALE = float(1.0 / np.sqrt(D))

BF16 = mybir.dt.bfloat16
F32 = mybir.dt.float32
F32R = mybir.dt.float32r
EXP = mybir.ActivationFunctionType.Exp

_CACHED = {}


def build():
    nc = bacc.Bacc("TRN2", target_bir_lowering=False, debug=False,
                   num_devices=NC)
    xT = nc.dram_tensor("xT", [C, T], BF16, kind="ExternalInput")
    wqkT = nc.dram_tensor("wqkT", [C, 4 * D], BF16, kind="ExternalInput")
    wvT = nc.dram_tensor("wvT", [C, DH], BF16, kind="ExternalInput")
    wpT = nc.dram_tensor("wpT", [DH, C], BF16, kind="ExternalInput")
    maskT = nc.dram_tensor("maskT", [128, 4 * NQ], BF16, kind="ExternalInput")
    out = nc.dram_tensor("out", [T, C], F32, kind="ExternalOutput")

    with tile.TileContext(nc) as tc:
        with (
            tc.tile_pool(name="const", bufs=1) as const,
            tc.tile_pool(name="work", bufs=4) as work,
            tc.tile_pool(name="psum", bufs=2, space="PSUM") as psum,
        ):
            # ---------------- input loads ----------------
            xT_sb = const.tile([128, KB, T], BF16)
            wqk_sb = const.tile([128, KB, 4 * D], BF16)
            wv_sb = const.tile([128, KB, DH], BF16)
            wp_sb = const.tile([128, HPC, C], BF16)
            mask_sb = const.tile([128, 4 * NQ], BF16)
            wqk_r = wqkT.ap().rearrange("(kb p) m -> p kb m", p=128)
            wv_r = wvT.ap().rearrange("(kb p) m -> p kb m", p=128)
            wp_r = wpT.ap().rearrange("(h p) j -> p h j", p=128)
            xT_r = xT.ap().rearrange("(kb p) t -> p kb t", p=128)
            # wqk first (small, gates every qk group), then all of x: no
            # qkv psum group can close before the last x k-tile lands, so
            # minimizing time-to-last-x-byte is what matters
            nc.sync.dma_start(wqk_sb[:, :, :], wqk_r)
            for g in range(4):
                nc.sync.dma_start(xT_sb[:, ts(g, 4), :], xT_r[:, ts(g, 4), :])
            nc.sync.dma_start(wv_sb[:, :, :], wv_r)
            nc.sync.dma_start(mask_sb[:, :], maskT[:, :])
            nc.sync.dma_start(wp_sb[:, :, :], wp_r)

            ones_col = const.tile([128, 1], BF16)
            nc.vector.memset(ones_col[:, :], 1.0)

            qk_sb = const.tile([128, 4, T], BF16)      # m: qh0 qh1 kh0 kh1
            v_sb = const.tile([128, TB, DH], BF16)     # v[tb] natural layout

            # ---------------- q,k projections (transposed) ----------------
            # first wave: 4 groups interleaved by kb-quarters so the PE's
            # in-order stream trickles 16 matmuls per arriving xT chunk
            # instead of stalling 4 matmuls into the first group
            wave = [(0, n) for n in range(QC)]
            wave_ps = {}
            for m, n in wave:
                wave_ps[(m, n)] = psum.tile([128, NQ], F32, tag="mm", bufs=4,
                                            name=f"ps_qk_{m}_{n}")
            for kbg in range(4):
                for m, n in wave:
                    for kb in range(4 * kbg, 4 * kbg + 4):
                        nc.tensor.matmul(
                            wave_ps[(m, n)][:, :],
                            wqk_sb[:, kb, ts(m, 128)],
                            xT_sb[:, kb, ts(n, NQ)],
                            start=(kb == 0), stop=(kb == KB - 1),
                        )
            for m, n in wave:
                nc.vector.tensor_copy(qk_sb[:, m, ts(n, NQ)],
                                      wave_ps[(m, n)][:, :])
            for m in (2, 1, 3):
                for n in range(QC):
                    ps = psum.tile([128, NQ], F32, tag="mm", bufs=4,
                                   name=f"ps_qk_{m}_{n}")
                    for kb in range(KB):
                        nc.tensor.matmul(
                            ps[:, :],
                            wqk_sb[:, kb, ts(m, 128)],
                            xT_sb[:, kb, ts(n, NQ)],
                            start=(kb == 0), stop=(kb == KB - 1),
                        )
                    nc.vector.tensor_copy(qk_sb[:, m, ts(n, NQ)], ps[:, :])

            # ---------------- v (natural layout) ----------------
            for tb in range(TB):
                psv = psum.tile([128, DH], F32, tag="mm", bufs=4,
                                name=f"ps_v_{tb}")
                for kb in range(KB):
                    nc.tensor.matmul(
                        psv[:, :],
                        xT_sb[:, kb, ts(tb, 128)],
                        wv_sb[:, kb, :],
                        start=(kb == 0), stop=(kb == KB - 1),
                    )
                nc.vector.tensor_copy(v_sb[:, tb, :], psv[:, :])

            # ------- attention (query-chunk major) + pipelined c_proj ------
            def emit_cproj(qc, oTs):
                # partial[t, j] += oT_h.T @ wp_h for this query chunk
                for tb in range(4):
                    fo = work.tile([128, QC, NQ], F32, tag="fo", bufs=4,
                                   name=f"fo_{qc}_{tb}")
                    for jn in range(QC):
                        psf = psum.tile([128, NQ], F32, tag="mm", bufs=4,
                                        name=f"ps_f_{qc}_{tb}_{jn}")
                        for h in range(HPC):
                            nc.tensor.matmul(
                                psf[:, :],
                                oTs[h][:, ts(tb, 128)],
                                wp_sb[:, h, ds(jn * NQ, NQ)],
                                start=(h == 0), stop=(h == HPC - 1),
                            )
                        if jn % 2 == 0:
                            nc.vector.tensor_copy(fo[:, jn, :], psf[:, :])
                        else:
                            nc.scalar.copy(fo[:, jn, :], psf[:, :])
                    nc.sync.dma_start(out[ds(qc * NQ + tb * 128, 128), :],
                                      fo[:, :, :])

            pending = None   # (qc, [oT_h0, oT_h1]) one chunk behind
            for qc in range(QC):
                norm_jobs = []
                for h in range(HPC):
                    qm, km = h, 2 + h
                    ps_o = psum.tile([128, NQ], F32, tag="o", bufs=3,
                                     name=f"ps_o_{h}_{qc}")
                    ps_l = psum.tile([1, NQ], F32, tag="l", bufs=1,
                                     name=f"ps_l_{h}_{qc}")
                    # diagonal blocks first: their longer exp+mask chain
                    # hides behind the remaining full-width ST stream
                    sb_order = list(range(4 * qc, 4 * qc + 4)) + \
                        list(range(4 * qc))
                    last = sb_order[-1]
                    first = sb_order[0]
                    for sb in sb_order:
                        r = sb - 4 * qc   # >= 0: diagonal region
                        off = 128 * r if r > 0 else 0
                        w = NQ - off
                        ps_s = psum.tile([128, NQ], F32, tag="mm", bufs=4,
                                         name=f"ps_s_{h}_{qc}_{sb}")
                        nc.tensor.matmul(
                            ps_s[:, off:NQ],
                            qk_sb[:, km, ts(sb, 128)],
                            qk_sb[:, qm, ds(qc * NQ + off, w)],
                            start=True, stop=True,
                        )
                        e = work.tile([128, NQ], BF16, tag="e", bufs=6,
                                      name=f"e_{h}_{qc}_{sb}")
                        nc.scalar.activation(e[:, off:NQ], ps_s[:, off:NQ],
                                             EXP, scale=SCALE)
                        if r >= 0:
                            nc.vector.tensor_mul(
                                e[:, off:NQ], e[:, off:NQ],
                                mask_sb[:, ds(r * NQ + off, w)])
                        nc.tensor.matmul(
                            ps_o[:, off:NQ],
                            v_sb[:, sb, ts(h, D)],
                            e[:, off:NQ],
                            start=(sb == first), stop=(sb == last),
                        )
                        nc.tensor.matmul(
                            ps_l[:, off:NQ],
                            ones_col[:, :],
                            e[:, off:NQ],
                            start=(sb == first), stop=(sb == last),
                        )
                    # free psums promptly; normalization chained off copies
                    o_sb = work.tile([128, NQ], F32, tag="o_sb", bufs=3,
                                     name=f"o_sb_{h}_{qc}")
                    nc.vector.tensor_copy(o_sb[:, :], ps_o[:, :])
                    rec = work.tile([1, NQ], F32, tag="rec", bufs=2,
                                    name=f"rec_{h}_{qc}")
                    nc.vector.reciprocal_approx_fast(rec[:, :], ps_l[:, :])
                    norm_jobs.append((h, o_sb, rec))
                oTs = []
                for h, o_sb, rec in norm_jobs:
                    bc = work.tile([128, NQ], F32, tag="bc", bufs=3,
                                   name=f"bc_{h}_{qc}")
                    nc.gpsimd.partition_broadcast(bc[:, :], rec[:, :])
                    oT = work.tile([128, NQ], BF16, tag="oT", bufs=6,
                                   name=f"oT_{h}_{qc}")
                    nc.vector.tensor_mul(oT[:, :], o_sb[:, :], bc[:, :])
                    oTs.append(oT)
                if pending is not None:
                    emit_cproj(*pending)
                pending = (qc, oTs)
            emit_cproj(*pending)

    nc.compile()
    return nc


def make_mask() -> np.ndarray:
    # mask[s, r*512 + t'] = 1 if t' >= 128*r + s  (key allowed for query)
    m = np.zeros((128, 4 * NQ), dtype=np.float32)
    s = np.arange(128)[:, None]
    tp = np.arange(NQ)[None, :]
    for r in range(4):
        m[:, r * NQ:(r + 1) * NQ] = (tp >= 128 * r + s)
    return m.astype(ml_dtypes.bfloat16)


def prep_inputs(x, W_attn, W_proj):
    bf = ml_dtypes.bfloat16
    xT_np = np.ascontiguousarray(x.T).astype(bf)
    mask_np = make_mask()
    Wq, Wk, Wv = W_attn[:C], W_attn[C:2 * C], W_attn[2 * C:]
    WpT = W_proj.T  # (C_in, C_out): [i, j]
    in_maps = []
    for c in range(NC):
        sl = slice(c * DH, (c + 1) * DH)
        wqk_c = np.concatenate([Wq[sl], Wk[sl]], axis=0)          # (512, C)
        wqkT_c = np.ascontiguousarray(wqk_c.T).astype(bf)          # (C, 512)
        wvT_c = np.ascontiguousarray(Wv[sl].T).astype(bf)          # (C, 256)
        wpT_c = np.ascontiguousarray(WpT[sl, :]).astype(bf)        # (256, C)
        in_maps.append({
            "xT": xT_np, "wqkT": wqkT_c, "wvT": wvT_c,
            "wpT": wpT_c, "maskT": mask_np,
        })
    return in_maps


def assemble(results) -> np.ndarray:
    acc = results[0]["out"].astype(np.float32, copy=True)
    for c in range(1, NC):
        acc += results[c]["out"]
    return acc


def kernel(x: np.ndarray, W_attn: np.ndarray, W_proj: np.ndarray) -> np.ndarray:
    x = np.asarray(x, dtype=np.float32)
    W_attn = np.asarray(W_attn, dtype=np.float32)
    W_proj = np.asarray(W_proj, dtype=np.float32)
    if "nc" not in _CACHED:
        _CACHED["nc"] = build()
    nc = _CACHED["nc"]
    in_maps = prep_inputs(x, W_attn, W_proj)
    try:
        res = run_bass_kernel_spmd(nc, in_maps, core_ids=list(range(NC)))
    except Exception:
        # rare transient device-unrecoverable states heal on retry
        res = run_bass_kernel_spmd(nc, in_maps, core_ids=list(range(NC)))
    return assemble(res.results)


# revision 27
# speedup vs baseline: 1.0215x; 1.0215x over previous
"""Causal self-attention (T=2048, C=2048, 16 heads) on 8 TRN2 NeuronCores.

Tensor-parallel over heads: core c owns heads 2c, 2c+1.
 - per-core qkv projection in transposed layout (qT,kT: [d, T]; v: [T, d])
 - scores computed transposed: ST[s, t] = kT_blk.T @ qT  (keys on partitions)
 - softmax without max-subtraction (scores are O(+-6) for randn inputs):
   E = exp(scale * ST) * causal_mask; denominator l via ones-matmul;
   out = (v.T @ E) / broadcast(l)  (K=1 matmul broadcast + DVE multiply)
 - no collectives: each core computes its PARTIAL final projection
   partial[t, j] = sum_{i in its 2 head sections} outT[i, t] * WpT[i, j],
   software-pipelined one query-chunk behind attention; the host sums the
   8 partials (the "all-reduce after c_proj" is done at host-gather time).
Host side: shard/transpose/cast inputs; sum the 8 partial outputs.
"""

import numpy as np
import ml_dtypes

import concourse.mybir as mybir
import concourse.tile as tile
from concourse import bacc
from concourse.bass import ds, ts
from concourse.bass_utils import run_bass_kernel_spmd

T = 2048
C = 2048
H = 16
D = 128            # head dim
NC = 8             # cores
HPC = H // NC      # heads per core
DH = HPC * D       # 256: qkv rows per section per core
KB = C // 128      # 16 contraction tiles
TB = T // 128      # 16 t tiles
NQ = 512           # query chunk (psum bank width)
QC = T // NQ       # 4 query chunks
SCALE = float(1.0 / np.sqrt(D))

BF16 = mybir.dt.bfloat16
F32 = mybir.dt.float32
F32R = mybir.dt.float32r
EXP = mybir.ActivationFunctionType.Exp

_CACHED = {}


def build():
    nc = bacc.Bacc("TRN2", target_bir_lowering=False, debug=False,
                   num_devices=NC)
    xT = nc.dram_tensor("xT", [C, T], BF16, kind="ExternalInput")
    wqkT = nc.dram_tensor("wqkT", [C, 4 * D], BF16, kind="ExternalInput")
    wvT = nc.dram_tensor("wvT", [C, DH], BF16, kind="ExternalInput")
    wpT = nc.dram_tensor("wpT", [DH, C], BF16, kind="ExternalInput")
    maskT = nc.dram_tensor("maskT", [128, 4 * NQ], BF16, kind="ExternalInput")
    out = nc.dram_tensor("out", [T, C], F32, kind="ExternalOutput")

    with tile.TileContext(nc) as tc:
        with (
            tc.tile_pool(name="const", bufs=1) as const,
            tc.tile_pool(name="work", bufs=4) as work,
            tc.tile_pool(name="psum", bufs=2, space="PSUM") as psum,
        ):
            # ---------------- input loads ----------------
            xT_sb = const.tile([128, KB, T], BF16)
            wqk_sb = const.tile([128, KB, 4 * D], BF16)
            wv_sb = const.tile([128, KB, DH], BF16)
            wp_sb = const.tile([128, HPC, C], BF16)
            mask_sb = const.tile([128, 4 * NQ], BF16)
            wqk_r = wqkT.ap().rearrange("(kb p) m -> p kb m", p=128)
            wv_r = wvT.ap().rearrange("(kb p) m -> p kb m", p=128)
            wp_r = wpT.ap().rearrange("(h p) j -> p h j", p=128)
            xT_r = xT.ap().rearrange("(kb p) t -> p kb t", p=128)
            # wqk first (small, gates every qk group), then all of x: no
            # qkv psum group can close before the last x k-tile lands, so
            # minimizing time-to-last-x-byte is what matters
            nc.sync.dma_start(wqk_sb[:, :, :], wqk_r)
            for g in range(4):
                nc.sync.dma_start(xT_sb[:, ts(g, 4), :], xT_r[:, ts(g, 4), :])
            nc.sync.dma_start(wv_sb[:, :, :], wv_r)
            nc.sync.dma_start(mask_sb[:, :], maskT[:, :])
            nc.sync.dma_start(wp_sb[:, :, :], wp_r)

            ones_col = const.tile([128, 1], BF16)
            nc.vector.memset(ones_col[:, :], 1.0)

            qk_sb = const.tile([128, 4, T], BF16)      # m: qh0 qh1 kh0 kh1
            v_sb = const.tile([128, TB, DH], BF16)     # v[tb] natural layout

            # ---------------- q,k projections (transposed) ----------------
            # first wave: 8 groups spread over ALL psum tags (attention's
            # o/l banks are idle in this phase), interleaved by kb-quarters,
            # so the PE's in-order stream has ~32 matmuls available per
            # arriving xT chunk -- enough to fully hide the input DMA
            wave = [(0, n) for n in range(QC)] + [(2, n) for n in range(QC)]
            wtags = [("mm", 4)] * 4 + [("o", 3)] * 3 + [("l", 1)]
            wave_ps = {}
            for (m, n), (tg, bf) in zip(wave, wtags):
                wave_ps[(m, n)] = psum.tile([128, NQ], F32, tag=tg, bufs=bf,
                                            name=f"ps_qk_{m}_{n}")
            for kbg in range(4):
                for m, n in wave:
                    for kb in range(4 * kbg, 4 * kbg + 4):
                        nc.tensor.matmul(
                            wave_ps[(m, n)][:, :],
                            wqk_sb[:, kb, ts(m, 128)],
                            xT_sb[:, kb, ts(n, NQ)],
                            start=(kb == 0), stop=(kb == KB - 1),
                        )
            for m, n in wave:
                nc.vector.tensor_copy(qk_sb[:, m, ts(n, NQ)],
                                      wave_ps[(m, n)][:, :])
            for m in (1, 3):
                for n in range(QC):
                    ps = psum.tile([128, NQ], F32, tag="mm", bufs=4,
                                   name=f"ps_qk_{m}_{n}")
                    for kb in range(KB):
                        nc.tensor.matmul(
                            ps[:, :],
                            wqk_sb[:, kb, ts(m, 128)],
                            xT_sb[:, kb, ts(n, NQ)],
                            start=(kb == 0), stop=(kb == KB - 1),
                        )
                    nc.vector.tensor_copy(qk_sb[:, m, ts(n, NQ)], ps[:, :])

            # ---------------- v (natural layout) ----------------
            for tb in range(TB):
                psv = psum.tile([128, DH], F32, tag="mm", bufs=4,
                                name=f"ps_v_{tb}")
                for kb in range(KB):
                    nc.tensor.matmul(
                        psv[:, :],
                        xT_sb[:, kb, ts(tb, 128)],
                        wv_sb[:, kb, :],
                        start=(kb == 0), stop=(kb == KB - 1),
                    )
                nc.vector.tensor_copy(v_sb[:, tb, :], psv[:, :])

            # ------- attention (query-chunk major) + pipelined c_proj ------
            def emit_cproj(qc, oTs):
                # partial[t, j] += oT_h.T @ wp_h for this query chunk
                for tb in range(4):
                    fo = work.tile([128, QC, NQ], F32, tag="fo", bufs=4,
                                   name=f"fo_{qc}_{tb}")
                    for jn in range(QC):
                        psf = psum.tile([128, NQ], F32, tag="mm", bufs=4,
                                        name=f"ps_f_{qc}_{tb}_{jn}")
                        for h in range(HPC):
                            nc.tensor.matmul(
                                psf[:, :],
                                oTs[h][:, ts(tb, 128)],
                                wp_sb[:, h, ds(jn * NQ, NQ)],
                                start=(h == 0), stop=(h == HPC - 1),
                            )
                        if jn % 2 == 0:
                            nc.vector.tensor_copy(fo[:, jn, :], psf[:, :])
                        else:
                            nc.scalar.copy(fo[:, jn, :], psf[:, :])
                    nc.sync.dma_start(out[ds(qc * NQ + tb * 128, 128), :],
                                      fo[:, :, :])

            pending = None   # (qc, [oT_h0, oT_h1]) one chunk behind
            for qc in range(QC):
                norm_jobs = []
                for h in range(HPC):
                    qm, km = h, 2 + h
                    ps_o = psum.tile([128, NQ], F32, tag="o", bufs=3,
                                     name=f"ps_o_{h}_{qc}")
                    ps_l = psum.tile([1, NQ], F32, tag="l", bufs=1,
                                     name=f"ps_l_{h}_{qc}")
                    # diagonal blocks first: their longer exp+mask chain
                    # hides behind the remaining full-width ST stream
                    sb_order = list(range(4 * qc, 4 * qc + 4)) + \
                        list(range(4 * qc))
                    last = sb_order[-1]
                    first = sb_order[0]
                    for sb in sb_order:
                        r = sb - 4 * qc   # >= 0: diagonal region
                        off = 128 * r if r > 0 else 0
                        w = NQ - off
                        ps_s = psum.tile([128, NQ], F32, tag="mm", bufs=4,
                                         name=f"ps_s_{h}_{qc}_{sb}")
                        nc.tensor.matmul(
                            ps_s[:, off:NQ],
                            qk_sb[:, km, ts(sb, 128)],
                            qk_sb[:, qm, ds(qc * NQ + off, w)],
                            start=True, stop=True,
                        )
                        e = work.tile([128, NQ], BF16, tag="e", bufs=6,
                                      name=f"e_{h}_{qc}_{sb}")
                        nc.scalar.activation(e[:, off:NQ], ps_s[:, off:NQ],
                                             EXP, scale=SCALE)
                        if r >= 0:
                            nc.vector.tensor_mul(
                                e[:, off:NQ], e[:, off:NQ],
                                mask_sb[:, ds(r * NQ + off, w)])
                        nc.tensor.matmul(
                            ps_o[:, off:NQ],
                            v_sb[:, sb, ts(h, D)],
                            e[:, off:NQ],
                            start=(sb == first), stop=(sb == last),
                        )
                        nc.tensor.matmul(
                            ps_l[:, off:NQ],
                            ones_col[:, :],
                            e[:, off:NQ],
                            start=(sb == first), stop=(sb == last),
                        )
                    # free psums promptly; normalization chained off copies
                    o_sb = work.tile([128, NQ], F32, tag="o_sb", bufs=3,
                                     name=f"o_sb_{h}_{qc}")
                    nc.vector.tensor_copy(o_sb[:, :], ps_o[:, :])
                    rec = work.tile([1, NQ], F32, tag="rec", bufs=2,
                                    name=f"rec_{h}_{qc}")
                    nc.vector.reciprocal_approx_fast(rec[:, :], ps_l[:, :])
                    norm_jobs.append((h, o_sb, rec))
                oTs = []
                for h, o_sb, rec in norm_jobs:
                    bc = work.tile([128, NQ], F32, tag="bc", bufs=3,
                                   name=f"bc_{h}_{qc}")
                    nc.gpsimd.partition_broadcast(bc[:, :], rec[:, :])
                    oT = work.tile([128, NQ], BF16, tag="oT", bufs=6,
                                   name=f"oT_{h}_{qc}")
                    nc.vector.tensor_mul(oT[:, :], o_sb[:, :], bc[:, :])
                    oTs.append(oT)
                if pending is not None:
                    emit_cproj(*pending)
                pending = (qc, oTs)
            emit_cproj(*pending)

    nc.compile()
    return nc


def make_mask() -> np.ndarray:
    # mask[s, r*512 + t'] = 1 if t' >= 128*r + s  (key allowed for query)
    m = np.zeros((128, 4 * NQ), dtype=np.float32)
    s = np.arange(128)[:, None]
    tp = np.arange(NQ)[None, :]
    for r in range(4):
        m[:, r * NQ:(r + 1) * NQ] = (tp >= 128 * r + s)
    return m.astype(ml_dtypes.bfloat16)


def prep_inputs(x, W_attn, W_proj):
    bf = ml_dtypes.bfloat16
    xT_np = np.ascontiguousarray(x.T).astype(bf)
    mask_np = make_mask()
    Wq, Wk, Wv = W_attn[:C], W_attn[C:2 * C], W_attn[2 * C:]
    WpT = W_proj.T  # (C_in, C_out): [i, j]
    in_maps = []
    for c in range(NC):
        sl = slice(c * DH, (c + 1) * DH)
        wqk_c = np.concatenate([Wq[sl], Wk[sl]], axis=0)          # (512, C)
        wqkT_c = np.ascontiguousarray(wqk_c.T).astype(bf)          # (C, 512)
        wvT_c = np.ascontiguousarray(Wv[sl].T).astype(bf)          # (C, 256)
        wpT_c = np.ascontiguousarray(WpT[sl, :]).astype(bf)        # (256, C)
        in_maps.append({
            "xT": xT_np, "wqkT": wqkT_c, "wvT": wvT_c,
            "wpT": wpT_c, "maskT": mask_np,
        })
    return in_maps


def assemble(results) -> np.ndarray:
    acc = results[0]["out"].astype(np.float32, copy=True)
    for c in range(1, NC):
        acc += results[c]["out"]
    return acc


def kernel(x: np.ndarray, W_attn: np.ndarray, W_proj: np.ndarray) -> np.ndarray:
    x = np.asarray(x, dtype=np.float32)
    W_attn = np.asarray(W_attn, dtype=np.float32)
    W_proj = np.asarray(W_proj, dtype=np.float32)
    if "nc" not in _CACHED:
        _CACHED["nc"] = build()
    nc = _CACHED["nc"]
    in_maps = prep_inputs(x, W_attn, W_proj)
    try:
        res = run_bass_kernel_spmd(nc, in_maps, core_ids=list(range(NC)))
    except Exception:
        # rare transient device-unrecoverable states heal on retry
        res = run_bass_kernel_spmd(nc, in_maps, core_ids=list(range(NC)))
    return assemble(res.results)


# revision 28
# speedup vs baseline: 1.0407x; 1.0189x over previous
"""Causal self-attention (T=2048, C=2048, 16 heads) on 8 TRN2 NeuronCores.

Tensor-parallel over heads: core c owns heads 2c, 2c+1.
 - per-core qkv projection in transposed layout (qT,kT: [d, T]; v: [T, d])
 - scores computed transposed: ST[s, t] = kT_blk.T @ qT  (keys on partitions)
 - softmax without max-subtraction (scores are O(+-6) for randn inputs):
   E = exp(scale * ST) * causal_mask; denominator l via ones-matmul;
   out = (v.T @ E) / broadcast(l)  (K=1 matmul broadcast + DVE multiply)
 - no collectives: each core computes its PARTIAL final projection
   partial[t, j] = sum_{i in its 2 head sections} outT[i, t] * WpT[i, j],
   software-pipelined one query-chunk behind attention; the host sums the
   8 partials (the "all-reduce after c_proj" is done at host-gather time).
Host side: shard/transpose/cast inputs; sum the 8 partial outputs.
"""

import numpy as np
import ml_dtypes

import concourse.mybir as mybir
import concourse.tile as tile
from concourse import bacc
from concourse.bass import ds, ts
from concourse.bass_utils import run_bass_kernel_spmd

T = 2048
C = 2048
H = 16
D = 128            # head dim
NC = 8             # cores
HPC = H // NC      # heads per core
DH = HPC * D       # 256: qkv rows per section per core
KB = C // 128      # 16 contraction tiles
TB = T // 128      # 16 t tiles
NQ = 512           # query chunk (psum bank width)
QC = T // NQ       # 4 query chunks
SCALE = float(1.0 / np.sqrt(D))

BF16 = mybir.dt.bfloat16
F32 = mybir.dt.float32
F32R = mybir.dt.float32r
EXP = mybir.ActivationFunctionType.Exp

_CACHED = {}


def build():
    nc = bacc.Bacc("TRN2", target_bir_lowering=False, debug=False,
                   num_devices=NC)
    xT = nc.dram_tensor("xT", [C, T], BF16, kind="ExternalInput")
    wqkT = nc.dram_tensor("wqkT", [C, 4 * D], BF16, kind="ExternalInput")
    wvT = nc.dram_tensor("wvT", [C, DH], BF16, kind="ExternalInput")
    wpT = nc.dram_tensor("wpT", [DH, C], BF16, kind="ExternalInput")
    maskT = nc.dram_tensor("maskT", [128, 4 * NQ], BF16, kind="ExternalInput")
    out = nc.dram_tensor("out", [T, C], F32, kind="ExternalOutput")

    with tile.TileContext(nc) as tc:
        with (
            tc.tile_pool(name="const", bufs=1) as const,
            tc.tile_pool(name="work", bufs=4) as work,
            tc.tile_pool(name="psum", bufs=2, space="PSUM") as psum,
        ):
            # ---------------- input loads ----------------
            xT_sb = const.tile([128, KB, T], BF16)
            wqk_sb = const.tile([128, KB, 4 * D], BF16)
            wv_sb = const.tile([128, KB, DH], BF16)
            wp_sb = const.tile([128, HPC, C], BF16)
            mask_sb = const.tile([128, 4 * NQ], BF16)
            wqk_r = wqkT.ap().rearrange("(kb p) m -> p kb m", p=128)
            wv_r = wvT.ap().rearrange("(kb p) m -> p kb m", p=128)
            wp_r = wpT.ap().rearrange("(h p) j -> p h j", p=128)
            xT_r = xT.ap().rearrange("(kb p) t -> p kb t", p=128)
            # wqk first (small, gates every qk group), then all of x: no
            # qkv psum group can close before the last x k-tile lands, so
            # minimizing time-to-last-x-byte is what matters
            # leading edge kb-granular (matches the wave's consumption
            # order so the PE starts within ~4us), bulk in 2MB chunks
            nc.sync.dma_start(wqk_sb[:, ts(0, 4), :], wqk_r[:, ts(0, 4), :])
            for kb in range(4):
                nc.sync.dma_start(xT_sb[:, kb, :], xT_r[:, kb, :])
            for g in range(1, 4):
                nc.sync.dma_start(wqk_sb[:, ts(g, 4), :],
                                  wqk_r[:, ts(g, 4), :])
                nc.sync.dma_start(xT_sb[:, ts(g, 4), :], xT_r[:, ts(g, 4), :])
            nc.sync.dma_start(wv_sb[:, :, :], wv_r)
            nc.sync.dma_start(mask_sb[:, :], maskT[:, :])
            nc.sync.dma_start(wp_sb[:, :, :], wp_r)

            ones_col = const.tile([128, 1], BF16)
            nc.vector.memset(ones_col[:, :], 1.0)

            qk_sb = const.tile([128, 4, T], BF16)      # m: qh0 qh1 kh0 kh1
            v_sb = const.tile([128, TB, DH], BF16)     # v[tb] natural layout

            # ---------------- q,k projections (transposed) ----------------
            # first wave: 8 groups spread over ALL psum tags (attention's
            # o/l banks are idle in this phase), interleaved by kb-quarters,
            # so the PE's in-order stream has ~32 matmuls available per
            # arriving xT chunk -- enough to fully hide the input DMA
            wave = [(0, n) for n in range(QC)] + [(2, n) for n in range(QC)]
            wtags = [("mm", 4)] * 4 + [("o", 3)] * 3 + [("l", 1)]
            wave_ps = {}
            for (m, n), (tg, bf) in zip(wave, wtags):
                wave_ps[(m, n)] = psum.tile([128, NQ], F32, tag=tg, bufs=bf,
                                            name=f"ps_qk_{m}_{n}")
            for kbg in range(4):
                for m, n in wave:
                    for kb in range(4 * kbg, 4 * kbg + 4):
                        nc.tensor.matmul(
                            wave_ps[(m, n)][:, :],
                            wqk_sb[:, kb, ts(m, 128)],
                            xT_sb[:, kb, ts(n, NQ)],
                            start=(kb == 0), stop=(kb == KB - 1),
                        )
            for m, n in wave:
                nc.vector.tensor_copy(qk_sb[:, m, ts(n, NQ)],
                                      wave_ps[(m, n)][:, :])
            for m in (1, 3):
                for n in range(QC):
                    ps = psum.tile([128, NQ], F32, tag="mm", bufs=4,
                                   name=f"ps_qk_{m}_{n}")
                    for kb in range(KB):
                        nc.tensor.matmul(
                            ps[:, :],
                            wqk_sb[:, kb, ts(m, 128)],
                            xT_sb[:, kb, ts(n, NQ)],
                            start=(kb == 0), stop=(kb == KB - 1),
                        )
                    nc.vector.tensor_copy(qk_sb[:, m, ts(n, NQ)], ps[:, :])

            # ---------------- v (natural layout) ----------------
            for tb in range(TB):
                psv = psum.tile([128, DH], F32, tag="mm", bufs=4,
                                name=f"ps_v_{tb}")
                for kb in range(KB):
                    nc.tensor.matmul(
                        psv[:, :],
                        xT_sb[:, kb, ts(tb, 128)],
                        wv_sb[:, kb, :],
                        start=(kb == 0), stop=(kb == KB - 1),
                    )
                nc.vector.tensor_copy(v_sb[:, tb, :], psv[:, :])

            # ------- attention (query-chunk major) + pipelined c_proj ------
            def emit_cproj(qc, oTs):
                # partial[t, j] += oT_h.T @ wp_h for this query chunk
                for tb in range(4):
                    fo = work.tile([128, QC, NQ], F32, tag="fo", bufs=4,
                                   name=f"fo_{qc}_{tb}")
                    for jn in range(QC):
                        psf = psum.tile([128, NQ], F32, tag="mm", bufs=4,
                                        name=f"ps_f_{qc}_{tb}_{jn}")
                        for h in range(HPC):
                            nc.tensor.matmul(
                                psf[:, :],
                                oTs[h][:, ts(tb, 128)],
                                wp_sb[:, h, ds(jn * NQ, NQ)],
                                start=(h == 0), stop=(h == HPC - 1),
                            )
                        if jn % 2 == 0:
                            nc.vector.tensor_copy(fo[:, jn, :], psf[:, :])
                        else:
                            nc.scalar.copy(fo[:, jn, :], psf[:, :])
                    nc.sync.dma_start(out[ds(qc * NQ + tb * 128, 128), :],
                                      fo[:, :, :])

            pending = None   # (qc, [oT_h0, oT_h1]) one chunk behind
            for qc in range(QC):
                norm_jobs = []
                for h in range(HPC):
                    qm, km = h, 2 + h
                    ps_o = psum.tile([128, NQ], F32, tag="o", bufs=3,
                                     name=f"ps_o_{h}_{qc}")
                    ps_l = psum.tile([1, NQ], F32, tag="l", bufs=1,
                                     name=f"ps_l_{h}_{qc}")
                    # diagonal blocks first: their longer exp+mask chain
                    # hides behind the remaining full-width ST stream
                    sb_order = list(range(4 * qc, 4 * qc + 4)) + \
                        list(range(4 * qc))
                    last = sb_order[-1]
                    first = sb_order[0]
                    for sb in sb_order:
                        r = sb - 4 * qc   # >= 0: diagonal region
                        off = 128 * r if r > 0 else 0
                        w = NQ - off
                        ps_s = psum.tile([128, NQ], F32, tag="mm", bufs=4,
                                         name=f"ps_s_{h}_{qc}_{sb}")
                        nc.tensor.matmul(
                            ps_s[:, off:NQ],
                            qk_sb[:, km, ts(sb, 128)],
                            qk_sb[:, qm, ds(qc * NQ + off, w)],
                            start=True, stop=True,
                        )
                        e = work.tile([128, NQ], BF16, tag="e", bufs=6,
                                      name=f"e_{h}_{qc}_{sb}")
                        nc.scalar.activation(e[:, off:NQ], ps_s[:, off:NQ],
                                             EXP, scale=SCALE)
                        if r >= 0:
                            nc.vector.tensor_mul(
                                e[:, off:NQ], e[:, off:NQ],
                                mask_sb[:, ds(r * NQ + off, w)])
                        nc.tensor.matmul(
                            ps_o[:, off:NQ],
                            v_sb[:, sb, ts(h, D)],
                            e[:, off:NQ],
                            start=(sb == first), stop=(sb == last),
                        )
                        nc.tensor.matmul(
                            ps_l[:, off:NQ],
                            ones_col[:, :],
                            e[:, off:NQ],
                            start=(sb == first), stop=(sb == last),
                        )
                    # free psums promptly; normalization chained off copies
                    o_sb = work.tile([128, NQ], F32, tag="o_sb", bufs=3,
                                     name=f"o_sb_{h}_{qc}")
                    nc.vector.tensor_copy(o_sb[:, :], ps_o[:, :])
                    rec = work.tile([1, NQ], F32, tag="rec", bufs=2,
                                    name=f"rec_{h}_{qc}")
                    nc.vector.reciprocal_approx_fast(rec[:, :], ps_l[:, :])
                    norm_jobs.append((h, o_sb, rec))
                oTs = []
                for h, o_sb, rec in norm_jobs:
                    bc = work.tile([128, NQ], F32, tag="bc", bufs=3,
                                   name=f"bc_{h}_{qc}")
                    nc.gpsimd.partition_broadcast(bc[:, :], rec[:, :])
                    oT = work.tile([128, NQ], BF16, tag="oT", bufs=6,
                                   name=f"oT_{h}_{qc}")
                    nc.vector.tensor_mul(oT[:, :], o_sb[:, :], bc[:, :])
                    oTs.append(oT)
                if pending is not None:
                    emit_cproj(*pending)
                pending = (qc, oTs)
            emit_cproj(*pending)

    nc.compile()
    return nc


def make_mask() -> np.ndarray:
    # mask[s, r*512 + t'] = 1 if t' >= 128*r + s  (key allowed for query)
    m = np.zeros((128, 4 * NQ), dtype=np.float32)
    s = np.arange(128)[:, None]
    tp = np.arange(NQ)[None, :]
    for r in range(4):
        m[:, r * NQ:(r + 1) * NQ] = (tp >= 128 * r + s)
    return m.astype(ml_dtypes.bfloat16)


def prep_inputs(x, W_attn, W_proj):
    bf = ml_dtypes.bfloat16
    xT_np = np.ascontiguousarray(x.T).astype(bf)
    mask_np = make_mask()
    Wq, Wk, Wv = W_attn[:C], W_attn[C:2 * C], W_attn[2 * C:]
    WpT = W_proj.T  # (C_in, C_out): [i, j]
    in_maps = []
    for c in range(NC):
        sl = slice(c * DH, (c + 1) * DH)
        wqk_c = np.concatenate([Wq[sl], Wk[sl]], axis=0)          # (512, C)
        wqkT_c = np.ascontiguousarray(wqk_c.T).astype(bf)          # (C, 512)
        wvT_c = np.ascontiguousarray(Wv[sl].T).astype(bf)          # (C, 256)
        wpT_c = np.ascontiguousarray(WpT[sl, :]).astype(bf)        # (256, C)
        in_maps.append({
            "xT": xT_np, "wqkT": wqkT_c, "wvT": wvT_c,
            "wpT": wpT_c, "maskT": mask_np,
        })
    return in_maps


def assemble(results) -> np.ndarray:
    acc = results[0]["out"].astype(np.float32, copy=True)
    for c in range(1, NC):
        acc += results[c]["out"]
    return acc


def kernel(x: np.ndarray, W_attn: np.ndarray, W_proj: np.ndarray) -> np.ndarray:
    x = np.asarray(x, dtype=np.float32)
    W_attn = np.asarray(W_attn, dtype=np.float32)
    W_proj = np.asarray(W_proj, dtype=np.float32)
    if "nc" not in _CACHED:
        _CACHED["nc"] = build()
    nc = _CACHED["nc"]
    in_maps = prep_inputs(x, W_attn, W_proj)
    try:
        res = run_bass_kernel_spmd(nc, in_maps, core_ids=list(range(NC)))
    except Exception:
        # rare transient device-unrecoverable states heal on retry
        res = run_bass_kernel_spmd(nc, in_maps, core_ids=list(range(NC)))
    return assemble(res.results)
